# revision 78
# baseline (speedup 1.0000x reference)
"""BiDAF-style attention kernel for Trainium2, data-parallel over batch on 8 cores.

Shapes (hardcoded): B=16, C=2048, Q=128, E=200, O=128. Each core: 2 batches.

Design (bf16 operands everywhere, fp32 psum accumulation, fp16 output):
- Scores computed ONCE, in cq-orientation ([c-part, q]) with bf16 matmuls
  (N=128 runs at 1 cycle/row in bf16; f32r would need N>=256). s_c folds
  into the rhs via lhsq = w3*xqT + w1; s_q rides the ones-row of xctb
  paired with an s_q row of lhsq_b (both at partition 64 so the s_q
  [1,128] psum->sbuf copy is partition-aligned, matmul tile_position
  rules permitting).
- Softmax-q: r = sum_q E via a DVE free-dim reduce over the exp'd tiles;
  1/r is a per-partition scalar in cq-layout so S1 = E*rinv is a cheap
  pool TT (no broadcast matmuls, no [1,C] reciprocals). S1^T comes from
  PE transposes (b0, while the DMA engines still stream inputs) or DMA
  xbar transposes (b1, when they're idle) - this kills the entire second
  score computation of the v1 kernel (~8k PE cycles/batch).
- Softmax-c side: t = S2^T Xc via ecq-lhsT matmuls against natural-layout
  context tiles; the xcn ones-column accumulates z. exp(s_c)/exp(s_q)
  factors cancel in the respective normalizations, so full scores are
  correct for both orientations.
- c2q's projection contribution goes through Y2 = Xq W2p^T (Q=O=128):
  the W2 chunks of the projection collapse from 2 matmuls to 1 per chunk,
  and c2q/q2c are only needed for the elementwise products, consumed
  straight from PSUM (DVE) or via an ACT bf16 copy + pool TT.
- Heavy software pipelining: all input DMAs are issued up front in
  need-order (HWDGE descgen is 625ns/DMA, serialized; xcn uses the Pool
  SWDGE path in parallel), b1's t-matmuls/tail and both Y2s are deferred
  into phase 2 of b0, and projections lag their chunk's products by two
  chunk-iterations so the psum->product->proj chains stay off the PE
  critical path.
"""

import numpy as np
import ml_dtypes

import concourse.bass as bass
import concourse.mybir as mybir
from concourse import bacc
from concourse.bass import MemorySpace
from concourse.masks import make_identity
from concourse.tile import TileContext
from concourse.bass_utils import run_bass_kernel_spmd

B, C, Q, E, O = 16, 2048, 128, 200, 128
NB = 2
NCORES = 8
NCT = 16          # c tiles of 128
F32 = mybir.dt.float32
F16 = mybir.dt.float16
BF = mybir.dt.bfloat16
EXP = mybir.ActivationFunctionType.Exp
MUL = mybir.AluOpType.mult
ADD = mybir.AluOpType.add

_CACHE = {}


def _build(num_devices=NCORES, reps=1):
    nc = bacc.Bacc("TRN2", target_bir_lowering=False, debug=False,
                   num_devices=num_devices)

    d_xcta = nc.dram_tensor("xcta", [NB, 128, C], BF, kind="ExternalInput")
    d_xctb = nc.dram_tensor("xctb", [NB, 73, C], BF, kind="ExternalInput")
    # natural-layout ctx tiles + ones col: [p, ct*201+j]
    d_xcn = nc.dram_tensor("xcn", [NB, 128, NCT * 201], BF,
                           kind="ExternalInput")
    d_xq = nc.dram_tensor("xq", [NB, 128, 466], BF, kind="ExternalInput")
    d_wp = nc.dram_tensor("wp", [128, 8 * O], BF, kind="ExternalInput")
    d_out = nc.dram_tensor("out_t", [NB, O, C], F16, kind="ExternalOutput")

    def mm(ps, lhsT, rhs, start=True, stop=True):
        nc.tensor.matmul(ps, lhsT, rhs, start=start, stop=stop)

    with TileContext(nc) as tc:
        with (
            tc.tile_pool(name="consts", bufs=1) as consts,
            tc.tile_pool(name="inputs", bufs=2) as inputs,
            tc.tile_pool(name="work", bufs=2) as work,
            tc.tile_pool(name="work1", bufs=1) as work1,
            tc.tile_pool(name="cqw", bufs=2, space=MemorySpace.PSUM) as ps_cqw,
            tc.tile_pool(name="pst", bufs=1, space=MemorySpace.PSUM) as ps_tp,
            tc.tile_pool(name="wa", bufs=3, space=MemorySpace.PSUM) as wa,
            tc.tile_pool(name="po", bufs=2, space=MemorySpace.PSUM) as po_pool,
        ):
            wp_all = consts.tile([128, 8, O], BF, tag="wp")
            ident = consts.tile([128, 128], BF, tag="ident")
            make_identity(nc, ident)

            for rep in range(reps):
                # ---- all input DMAs up front on one ordered HWDGE stream;
                # order matches the time each chunk is first needed (the
                # 625ns/DMA serialized descgen is the early pacer) ----
                xqs, xctas, xctbs, xcns = [], [], [], []
                for b in range(NB):
                    xq = inputs.tile([128, 466], BF, tag="xq")
                    xcta = inputs.tile([128, C], BF, tag="xcta")
                    xctb = inputs.tile([73, C], BF, tag="xctb")
                    xcn = inputs.tile([128, NCT, 201], BF, tag="xcn")
                    xqs.append(xq)
                    xctas.append(xcta)
                    xctbs.append(xctb)
                    xcns.append(xcn)

                def in_half(b, h):
                    csl = slice(h * 1024, (h + 1) * 1024)
                    nc.sync.dma_start(out=xctas[b][:, csl],
                                      in_=d_xcta.ap()[b][:, csl])
                    nc.sync.dma_start(out=xctbs[b][:, csl],
                                      in_=d_xctb.ap()[b][:, csl])

                # SP/HWDGE stream carries only what phase 1(b0) + early
                # phase 1(b1) needs (SP SEQ costs 650ns per DMA, serially);
                # xcn goes through the Pool SWDGE path in parallel. xcn(b1)
                # and wp are emitted later (inside body b0) so their
                # transfers don't cut ahead of more urgent chunks.
                nc.sync.dma_start(out=xqs[0][:, 0:264],
                                  in_=d_xq.ap()[0][:, 0:264])
                in_half(0, 0)
                nc.sync.dma_start(out=xcns[0][:, 0:4, :],
                                  in_=d_xcn.ap()[0][:, 0:804])
                in_half(0, 1)
                nc.sync.dma_start(out=xcns[0][:, 4:16, :],
                                  in_=d_xcn.ap()[0][:, 804:3216])
                nc.sync.dma_start(out=xqs[1][:, 0:264],
                                  in_=d_xq.ap()[1][:, 0:264])
                nc.sync.dma_start(out=xctas[1], in_=d_xcta.ap()[1])
                nc.sync.dma_start(out=xctbs[1], in_=d_xctb.ap()[1])
                nc.sync.dma_start(out=xqs[0][:, 264:465],
                                  in_=d_xq.ap()[0][:, 264:465])
                if rep == 0:
                    nc.sync.dma_start(
                        out=wp_all,
                        in_=d_wp.ap().rearrange("p (k o) -> p k o", k=8))
                nc.sync.dma_start(out=xqs[1][:, 264:465],
                                  in_=d_xq.ap()[1][:, 264:465])
                nc.sync.dma_start(out=xcns[1], in_=d_xcn.ap()[1])

                # ---- phase 1 prologues: lhsq stt for both batches early
                # (they only need the xq/wc DMAs); b1's s_q matmuls stay in
                # its body so they don't block b0's PE work ----
                lhsqs = []
                for b in range(NB):
                    xq = xqs[b]
                    lhsq_a = work.tile([128, 128], BF, tag="lhsq_a")
                    nc.vector.scalar_tensor_tensor(
                        lhsq_a, xq[:, 0:128],
                        xq[:, 260:262].bitcast(F32),
                        xq[:, 258:259].broadcast_to([128, 128]),
                        op0=MUL, op1=ADD)
                    lhsq_b = work.tile([73, 128], BF, tag="lhsq_b")
                    nc.vector.scalar_tensor_tensor(
                        lhsq_b, xq[0:73, 128:256],
                        xq[0:73, 262:264].bitcast(F32),
                        xq[0:73, 259:260].broadcast_to([73, 128]),
                        op0=MUL, op1=ADD)
                    lhsqs.append((lhsq_a, lhsq_b))

                # ---- phase 1 bodies: scores, softmax pieces, t ----
                # b1's t-matmuls and both Y2s are deferred past its group
                # loop (their xcn/wp DMAs land late); the DVE/pool tail of
                # b1 (s1 half 1, rz, tsb) is deferred into b0's phase 2 so
                # it doesn't sit in front of b0's product TTs.
                s1ts, tsbs, tails, t_groups, xposes, y2s = ([], [], [],
                                                             [], [], [])
                for b in range(NB):
                    xq, xcta, xctb, xcn = xqs[b], xctas[b], xctbs[b], xcns[b]
                    lhsq_a, lhsq_b = lhsqs[b]
                    # s_q row -> psum partition 64 -> lhsq_b row 64
                    ps_sq = po_pool.tile([128, 512], F32, tag="po")
                    mm(ps_sq[64:65, 0:128], xq[:, 256:257], xq[:, 0:128],
                       start=True, stop=False)
                    mm(ps_sq[64:65, 0:128], xq[0:73, 257:258],
                       xq[0:73, 128:256], start=False, stop=True)
                    if b == 0:
                        nc.vector.tensor_copy(lhsq_b[64:65, :],
                                              ps_sq[64:65, 0:128])
                    else:
                        nc.scalar.copy(lhsq_b[64:65, :],
                                       ps_sq[64:65, 0:128])

                    ecq = work.tile([128, NCT, 128], BF, tag="ecq")
                    rcol = work.tile([128, NCT], F32, tag="rcol")
                    rinv = work.tile([128, NCT, 1], F32, tag="rinv")
                    s1 = work.tile([128, NCT, 128], BF, tag="s1")
                    s1t = work.tile([128, NCT, 128], BF, tag="s1t")
                    ps_t = ps_tp.tile([128, 512], F32, tag="pst")
                    rz = work.tile([128, 1], F32, tag="rz")
                    tsb = work.tile([128, 224], BF, tag="tsb")

                    def pe_xpose(qt, s1=s1, s1t=s1t):
                        # PE transpose + copy: used for b0's quarters, whose
                        # DMA-transpose would sit behind the congested input
                        # stream; emitted inside b1's group loop so the PE
                        # wait on the s1 pool TT doesn't block b0's groups
                        tsl = slice(qt * 4, (qt + 1) * 4)
                        pt = wa.tile([128, 512], F32, tag="wa")
                        ptb = pt.bitcast(BF)
                        for k in range(4):
                            nc.tensor.transpose(
                                ptb[:, k * 128:(k + 1) * 128],
                                s1[:, qt * 4 + k, :], ident)
                        nc.scalar.copy(s1t[:, tsl, :], ptb[:, 0:512])

                    def s1_quarter(qt, b=b, rinv=rinv, s1=s1, ecq=ecq,
                                   s1t=s1t, pe_xpose=pe_xpose):
                        tsl = slice(qt * 4, (qt + 1) * 4)
                        nc.gpsimd.tensor_mul(
                            s1[:, tsl, :], ecq[:, tsl, :],
                            rinv[:, tsl, :].broadcast_to([128, 4, 128]))
                        if b == 0:
                            xposes.append(lambda qt=qt: pe_xpose(qt))
                        else:
                            nc.sync.dma_start_transpose(
                                out=s1t[:, tsl, :], in_=s1[:, tsl, :])

                    def s1_half(h, rinv=rinv, rcol=rcol,
                                s1_quarter=s1_quarter):
                        tsl = slice(h * 8, (h + 1) * 8)
                        nc.vector.reciprocal(rinv[:, tsl, 0], rcol[:, tsl])
                        s1_quarter(2 * h)
                        s1_quarter(2 * h + 1)

                    def t_group(g, ecq=ecq, xcn=xcn, ps_t=ps_t):
                        for ct in range(4 * g, 4 * g + 4):
                            mm(ps_t[:, 0:201], ecq[:, ct, :], xcn[:, ct, :],
                               start=(ct == 0), stop=(ct == NCT - 1))

                    for g in range(4):
                        gsl = slice(g * 4, (g + 1) * 4)
                        ps = ps_cqw.tile([128, 512], F32, tag="cqw")
                        for k in range(4):
                            ct = 4 * g + k
                            csl = slice(ct * 128, (ct + 1) * 128)
                            ksl = slice(k * 128, (k + 1) * 128)
                            mm(ps[:, ksl], xcta[:, csl], lhsq_a,
                               start=True, stop=False)
                            mm(ps[:, ksl], xctb[:, csl], lhsq_b,
                               start=False, stop=True)
                        nc.scalar.activation(
                            out=ecq[:, gsl, :], in_=ps, func=EXP)
                        nc.vector.reduce_sum(
                            rcol[:, gsl], ecq[:, gsl, :],
                            axis=mybir.AxisListType.X)
                        if b == 1 and g < min(2, len(xposes)):
                            xposes[g]()
                        if b == 0 and g > 0:
                            t_group(g - 1)
                        if g == 1:
                            if b == 0:
                                tsl = slice(0, 8)
                                nc.vector.reciprocal(rinv[:, tsl, 0],
                                                     rcol[:, tsl])
                                s1_quarter(0)
                            else:
                                s1_half(0)
                    if b == 0:
                        t_group(3)
                        t_groups.append(None)
                    else:
                        t_groups.append(t_group)

                    def tail(b=b, ps_t=ps_t, rz=rz, tsb=tsb,
                             s1_half=s1_half, s1_quarter=s1_quarter,
                             rinv=rinv, rcol=rcol):
                        # t = S2^T Xc (exp(s_q) factor cancels via z col
                        # 200); tsb first - phase 2's q2c matmuls need it
                        # and it must not queue behind the s1 DVE work
                        nc.vector.reciprocal(rz, ps_t[:, 200:201])
                        nc.vector.memset(tsb[:, 192:193], 0.0)
                        nc.vector.tensor_scalar_mul(
                            tsb[:, 0:192], ps_t[:, 0:192], rz)
                        nc.vector.tensor_scalar_mul(
                            tsb[:, 193:201], ps_t[:, 192:200], rz)
                        if b == 0:
                            s1_quarter(1)
                            nc.vector.reciprocal(rinv[:, 8:16, 0],
                                                 rcol[:, 8:16])
                            s1_quarter(2)
                            s1_quarter(3)
                        else:
                            s1_half(1)

                    if b == 0:
                        tail()
                        tails.append(None)
                    else:
                        tails.append(tail)
                    s1ts.append(s1t)
                    tsbs.append(tsb)

                for b in range(NB):
                    xq = xqs[b]
                    ps_y2 = ps_cqw.tile([128, 512], F32, tag="cqw")
                    mm(ps_y2[:, 0:128], xq[:, 0:128], wp_all[:, 2, :],
                       start=True, stop=False)
                    mm(ps_y2[:, 0:128], xq[0:73, 128:256],
                       wp_all[0:73, 3, :], start=False, stop=True)
                    y2 = work.tile([128, 128], BF, tag="y2")
                    nc.scalar.copy(y2, ps_y2[:, 0:128])
                    y2s.append(y2)

                # ---- phase 2: chunks of both batches interleaved so
                # every chunk's products get ~2 PE-iterations of slack
                # before its projection reads them ----
                p1as, p1bs, p2as, p2bs, q2bs, out_sbs = {}, {}, {}, {}, {}, {}
                for b in range(NB):
                    p1a = work1.tile([128, C], BF, tag=f"p1a{b}")
                    p1b = work1.tile([73, C], BF, tag=f"p1b{b}")
                    p2a = work1.tile([128, C], BF, tag=f"p2a{b}")
                    p2b = work1.tile([73, C], BF, tag=f"p2b{b}")
                    q2b = work1.tile([73, C], BF, tag=f"q2b{b}")
                    out_sb = work1.tile([O, C], F16, tag=f"osb{b}")
                    p1as[b], p1bs[b], p2as[b] = p1a, p1b, p2a
                    p2bs[b], q2bs[b], out_sbs[b] = p2b, q2b, out_sb

                def emit_products(b, ch):
                    xq, xcta, xctb = xqs[b], xctas[b], xctbs[b]
                    s1t, tsb = s1ts[b], tsbs[b]
                    csl = slice(ch * 512, (ch + 1) * 512)
                    s1t_ch = s1t[:, 4 * ch:4 * ch + 4, :]
                    # q2c-b first: deepest chain (ACT copy -> pool TT)
                    pb2 = wa.tile([128, 512], F32, tag="wa")
                    mm(pb2[0:73, :], tsb[:, 128:201], s1t_ch)
                    nc.scalar.copy(q2bs[b][0:73, csl], pb2[0:73, :])
                    nc.gpsimd.tensor_mul(p2bs[b][0:73, csl],
                                         q2bs[b][0:73, csl], xctb[:, csl])
                    pa1 = wa.tile([128, 512], F32, tag="wa")
                    mm(pa1, xq[:, 264:392], s1t_ch)
                    pb1 = wa.tile([128, 512], F32, tag="wa")
                    mm(pb1[0:73, :], xq[:, 392:465], s1t_ch)
                    pa2 = po_pool.tile([128, 512], F32, tag="po")
                    mm(pa2, tsb[:, 0:128], s1t_ch)
                    # p1/p2a straight from psum on DVE
                    nc.vector.tensor_mul(p1as[b][:, csl], pa1, xcta[:, csl])
                    nc.vector.tensor_mul(p1bs[b][0:73, csl], pb1[0:73, :],
                                         xctb[:, csl])
                    nc.vector.tensor_mul(p2as[b][:, csl], pa2, xcta[:, csl])

                def emit_proj(b, ch):
                    xcta, xctb = xctas[b], xctbs[b]
                    y2, s1t = y2s[b], s1ts[b]
                    csl = slice(ch * 512, (ch + 1) * 512)
                    s1t_ch = s1t[:, 4 * ch:4 * ch + 4, :]
                    pp = po_pool.tile([128, 512], F32, tag="po")
                    mm(pp, wp_all[:, 0, :], xcta[:, csl],
                       start=True, stop=False)
                    mm(pp, wp_all[0:73, 1, :], xctb[:, csl],
                       start=False, stop=False)
                    mm(pp, y2, s1t_ch, start=False, stop=False)
                    mm(pp, wp_all[:, 4, :], p1as[b][:, csl],
                       start=False, stop=False)
                    mm(pp, wp_all[0:73, 5, :], p1bs[b][0:73, csl],
                       start=False, stop=False)
                    mm(pp, wp_all[:, 6, :], p2as[b][:, csl],
                       start=False, stop=False)
                    mm(pp, wp_all[0:73, 7, :], p2bs[b][0:73, csl],
                       start=False, stop=True)
                    nc.scalar.copy(out_sbs[b][:, csl], pp)
                    nc.sync.dma_start(out=d_out.ap()[b][:, csl],
                                      in_=out_sbs[b][:, csl])

                seq = [(0, 0), (0, 1), (0, 2), (0, 3),
                       (1, 0), (1, 1), (1, 2), (1, 3)]
                for k, (b, ch) in enumerate(seq):
                    emit_products(b, ch)
                    if k in (1, 2) and len(xposes) > k + 1:
                        xposes[k + 1]()
                    if k in (0, 1) and t_groups[1]:
                        for g in (2 * k, 2 * k + 1):
                            t_groups[1](g)
                        if k == 1:
                            t_groups[1] = None
                    if k >= 2:
                        emit_proj(*seq[k - 2])
                    if k == 1 and tails[1]:
                        tails[1]()
                        tails[1] = None
                emit_proj(*seq[6])
                emit_proj(*seq[7])

    nc.compile()
    return nc


def _get_nc():
    if "nc" not in _CACHE:
        _CACHE["nc"] = _build()
    return _CACHE["nc"]


def _pack_rearranged(dst, src, row64=None):
    """dst rows 0:64 = src rows 0:64; row 64 = row64 (or 0); 65:73 = src 64:72."""
    dst[0:64] = src[0:64]
    if row64 is not None:
        dst[64] = row64
    dst[65:73] = src[64:72]


def kernel(x_contexts, x_questions, w_sim, w_proj, b_proj, _trace=False):
    bf16 = ml_dtypes.bfloat16
    x_contexts = np.ascontiguousarray(x_contexts, dtype=np.float32)
    x_questions = np.ascontiguousarray(x_questions, dtype=np.float32)
    w_sim = np.asarray(w_sim, dtype=np.float32)
    w_proj = np.asarray(w_proj, dtype=np.float32)
    b_proj = np.asarray(b_proj, dtype=np.float32)
    w1, w2, w3 = w_sim[0, 0:E], w_sim[0, E:2 * E], w_sim[0, 2 * E:]

    xct = x_contexts.transpose(0, 2, 1)            # [B, E, C]
    xcta = np.ascontiguousarray(xct[:, 0:128]).astype(bf16)
    xctb = np.zeros((B, 73, C), np.float32)
    for bi in range(B):
        _pack_rearranged(xctb[bi], xct[bi, 128:200], row64=1.0)
    xctb = xctb.astype(bf16)
    xcn = np.zeros((B, 128, NCT, 201), np.float32)
    xcn[:, :, :, 0:E] = x_contexts.reshape(B, NCT, 128, E).transpose(0, 2, 1, 3)
    xcn[:, :, :, E] = 1.0
    xcn = xcn.reshape(B, 128, NCT * 201).astype(bf16)

    xqt = x_questions.transpose(0, 2, 1)           # [B, E, Q]
    xq = np.zeros((B, 128, 466), np.float32)
    xq[:, :, 0:128] = xqt[:, 0:128]
    xq[:, 0:64, 128:256] = xqt[:, 128:192]
    xq[:, 65:73, 128:256] = xqt[:, 192:200]
    xq[:, 0:128, 256] = w2[0:128]
    xq[:, 0:64, 257] = w2[128:192]
    xq[:, 65:73, 257] = w2[192:200]
    xq[:, 0:128, 258] = w1[0:128]
    xq[:, 0:64, 259] = w1[128:192]
    xq[:, 65:73, 259] = w1[192:200]
    xq[:, :, 264:392] = x_questions[:, :, 0:128]
    xq[:, :, 392:456] = x_questions[:, :, 128:192]
    xq[:, :, 456] = 0.0
    xq[:, :, 457:465] = x_questions[:, :, 192:200]
    xq = xq.astype(bf16)
    # raw f32 w3 columns, carried through bf16 slots via bitcast (cols
    # 260:262 = w3a, 262:264 = w3b-rearranged)
    w3a_u = w3[0:128].astype("<f4").view("<u2").reshape(128, 2)
    w3b_f = np.zeros(73, np.float32)
    w3b_f[0:64] = w3[128:192]
    w3b_f[65:73] = w3[192:200]
    w3b_u = w3b_f.astype("<f4").view("<u2").reshape(73, 2)
    xq_u = xq.view(np.uint16)
    xq_u[:, :, 260:262] = w3a_u[None, :, :]
    xq_u[:, 0:73, 262:264] = w3b_u[None, :, :]

    wpT = w_proj.T                                 # [800, O]
    wp = np.zeros((8, 128, O), np.float32)
    wp[0] = wpT[0:128]                             # W1 e0:128
    _pack_rearranged(wp[1], wpT[128:200], row64=None)
    wp[1, 64] = b_proj                             # bias pairs the ones row
    wp[2] = wpT[200:328]                           # W2^T e0:128 (Y2 rhs)
    _pack_rearranged(wp[3], wpT[328:400])
    wp[4] = wpT[400:528]                           # W3 e0:128
    _pack_rearranged(wp[5], wpT[528:600])
    wp[6] = wpT[600:728]                           # W4 e0:128
    _pack_rearranged(wp[7], wpT[728:800])
    # device layout [p, k*O+o]: per-partition rows of 8*O contiguous bf16
    wp = np.ascontiguousarray(wp.transpose(1, 0, 2).reshape(128, 8 * O)
                              ).astype(bf16)

    in_maps = []
    for c in range(NCORES):
        bs = slice(c * NB, (c + 1) * NB)
        in_maps.append({
            "xcta": np.ascontiguousarray(xcta[bs]),
            "xctb": np.ascontiguousarray(xctb[bs]),
            "xcn": np.ascontiguousarray(xcn[bs]),
            "xq": np.ascontiguousarray(xq[bs]),
            "wp": wp,
        })

    nc = _get_nc()
    res = run_bass_kernel_spmd(nc, in_maps, core_ids=list(range(NCORES)),
                               trace=_trace)
    _CACHE["last_res"] = res

    out = np.empty((B, C, O), np.float32)
    for c in range(NCORES):
        ot = res.results[c]["out_t"]               # [NB, O, C] f32
        for b in range(NB):
            out[c * NB + b] = np.asarray(ot[b], dtype=np.float32).T
    return out
